# revision 5
# baseline (speedup 1.0000x reference)
"""Distributed TRN2 Bass kernel for nn_Att_scores (attention score double-sum).

Math: the reference computes
    qkv = X @ W_qkv.T ; q, k = split ; attn = (q @ k^T) * scale
    scores = attn.sum(heads).sum(keys)                          # [B, N]
Both sums commute with the matmuls, so no [N, N] attention matrix is needed:
    s[b]        = sum_n X[b, n, :]          (row-sum of X)       # [C]
    t[b]        = Wk @ s[b]                                      # [C]
    u[b]        = Wq^T @ t[b]                                    # [C]
    scores[b,n] = scale * X[b, n, :] . u[b]
i.e. one global row-sum, two 768x768 matvecs, one per-row dot.

Distribution: shard the CHANNEL axis across the 8 cores (96 channels each).
Core i holds X^T[ci, :, :] (its channels, ALL rows), Wk[:, ci]^T, Wq[:, ci]:
    s_i = sum_n X[b, n, ci]                 local (channel-complete, no AR)
    t_part_i = Wk[:, ci] @ s_i              local rank-96 partial of t
    AllGather(t_part, 3 KB/rank)            the ONLY collective
    u[ci] = Wq[:, ci]^T t                   the 8-way t-sum folds into the
                                            u-matmul free dim (f=16) + two
                                            tiny strided DVE reduces
    ps_i[b, n] = sum_{c in ci} X[b,n,c]u[c] partial scores over ALL rows
    host: scores = sum_i ps_i               unshard-by-sum in assemble_out
Per-core HBM traffic ~1.1 MB/iter (bf16): X slice 0.79 MB once, two 96-col
W blocks, one 3 KB gather, 16 KB store.  (The v1 baseline moved ~9.4 MB and
was pure-DMA-bound at ~20 us.)

Precision: host pre-casts the sharded inputs to bf16 (the kernel used SWDGE
cast-DMAs fp32->bf16 on the same values anyway, so device numerics are
unchanged); all accumulation is fp32 on PSUM; the AllGather payload is the
bf16 t-partials.  Measured absmax relative error vs the fp32 reference:
5.5e-3 (gate 2e-2).

Engine mapping / scheduling:
  * bulk loads ride the Sync HWDGE ring (2 X slabs + 2 W blocks, 2 KB/desc
    per partition; finer descriptors measurably LOSE to the sub-512B HBM
    penalty)
  * s_i: free-axis reduction split DVE reduce_sum (b=0) / ACT Copy+accum_out
    (b=1)
  * t_part: s-stationary TensorE matmul; drain split ACT/DVE
  * gather bounce + gathered-t load ride the ScalarE HWDGE ring; output
    stores are SWDGE cast-DMAs bf16->fp32 on the (otherwise idle) Pool queue
  * t^T: 6 TensorE transposes of the gathered [16, 768] + DVE copies;
    u: Wq-stationary matmul (f=16); SCALE folds into the uT PSUM drain
  * partial scores: 8 u-stationary [2, 512] matmuls, PSUM tag rotated 3-deep
    (bufs=3 was worth ~2 us: the drain chain was PSUM-buffer-limited);
    drains alternate ACT/DVE
  * engines execute in order, so the unrolled reps are emitted as a software
    pipeline with lag=5: block k = pre-stage(k) [loads, s, t_part, gather]
    + post-stage(k-5) [gathered load, t^T, u, scores, store].  Per-rep SBUF
    tiles rotate with bufs=lag+2.  Without this the collective's ~9 us
    round-trip latency lands on the TensorE critical path every rep.
  * AllGather beats AllReduce here (~2 us/iter measured): half the ring
    occupancy, and the sum it skips is nearly free on-device.
"""

import numpy as np

B = 2
N = 2048
C = 768
H = 12
HD = C // H
SCALE = float(HD) ** -0.5
NCORES = 8
JB = C // NCORES          # 96 channels per core
JT = C // 128             # 6 chunks of 128 rows of Wq
NSEG = (B * N) // 512     # 8 output segments of 512 columns

_compiled_nc = None


def _build_and_compile(use_collective=True, repeats=1, warm=0, lag=5, ar_unused=False, ar_indep=False, nslab=2):
    import concourse.bass as bass  # noqa: F401
    import concourse.bacc as bacc
    import concourse.tile as tile
    import concourse.mybir as mybir
    from concourse import masks

    f32 = mybir.dt.float32
    bf16 = mybir.dt.bfloat16
    add = mybir.AluOpType.add
    copy_fn = mybir.ActivationFunctionType.Copy
    AX = mybir.AxisListType.X

    nc = bacc.Bacc(
        "TRN2",
        target_bir_lowering=False,
        debug=False,
        num_devices=NCORES,
    )

    x_d = nc.dram_tensor("x_in", [JB, B * N], bf16, kind="ExternalInput")
    wkt_d = nc.dram_tensor("wkt_in", [JB, C], bf16, kind="ExternalInput")
    wq_d = nc.dram_tensor("wq_in", [128, JT * JB], bf16, kind="ExternalInput")
    out_d = nc.dram_tensor("scores_out", [B, N], f32, kind="ExternalOutput")

    SLICES = ((0, 512), (512, 256))
    lag = min(lag, max(repeats - 1, 0))
    sbufs = lag + 2

    with tile.TileContext(nc) as tc:
        with (
            tc.tile_pool(name="sbuf", bufs=1) as pool,
            tc.tile_pool(name="psum", bufs=1, space="PSUM") as psum,
            tc.tile_pool(name="dram", bufs=1, space="DRAM") as dram,
        ):
            ones_red = pool.tile([128, 1], bf16)
            ident_g = pool.tile([2 * NCORES, 2 * NCORES], bf16)
            nc.gpsimd.memset(ones_red[:], 1.0)
            masks.make_identity(nc, ident_g[:])
            ar_static = dram.tile([1, B, C], bf16, name="ar_static")
            if ar_indep:
                zz = pool.tile([1, B * C], bf16, name="zz")
                nc.gpsimd.memset(zz[:], 0.0)
                nc.scalar.dma_start(
                    ar_static[:].rearrange("x b c -> x (b c)"), zz[:]
                )

            state = {}

            def pre_stage(rep):
                xc_sb = pool.tile([JB, B * N], bf16, tag="xc", bufs=sbufs)
                scratch = pool.tile([JB, 2048], bf16, tag="scr", bufs=2)
                wkt_sb = pool.tile([JB, C], bf16, tag="wkt", bufs=sbufs)
                wqc_sb = pool.tile([128, JT, JB], bf16, tag="wqc", bufs=sbufs)
                s_f = pool.tile([JB, 4], f32, tag="sf", bufs=2)
                s_bf = pool.tile([JB, B], bf16, tag="sbf", bufs=2)
                tp_sb = pool.tile([B, C], bf16, tag="tp", bufs=2)
                t_full = pool.tile([B * NCORES, C], bf16, tag="tf", bufs=sbufs)

                slab = (B * N) // nslab
                for q in range(nslab):
                    nc.sync.dma_start(
                        xc_sb[:, q * slab : (q + 1) * slab],
                        x_d[:, q * slab : (q + 1) * slab],
                    )
                nc.sync.dma_start(wkt_sb[:], wkt_d.ap())
                nc.sync.dma_start(
                    wqc_sb[:], wq_d.ap().rearrange("p (t f) -> p t f", t=JT)
                )

                nc.vector.reduce_sum(s_f[:, 0:1], xc_sb[:, 0:2048], axis=AX)
                nc.scalar.activation(
                    scratch[:, 0:2048], xc_sb[:, 2048:4096],
                    copy_fn, accum_out=s_f[:, 2:3],
                )
                nc.vector.tensor_copy(s_bf[:, 0:1], s_f[:, 0:1])
                nc.vector.tensor_copy(s_bf[:, 1:2], s_f[:, 2:3])

                tp_ps = psum.tile([B, 1024], f32, tag="tp", bufs=1, name="tp")
                for lo, nsz in SLICES:
                    nc.tensor.matmul(
                        tp_ps[:, lo : lo + nsz],
                        s_bf[:],
                        wkt_sb[:, lo : lo + nsz],
                        start=True,
                        stop=True,
                    )
                nc.scalar.copy(tp_sb[:, 0:512], tp_ps[:, 0:512])
                nc.vector.tensor_copy(tp_sb[:, 512:C], tp_ps[:, 512:C])

                ar_in = dram.tile([1, B, C], bf16, name=f"ar_in{rep}")
                ar_out = dram.tile(
                    [NCORES, B, C], bf16, addr_space="Shared", name=f"ar_out{rep}"
                )
                nc.scalar.dma_start(
                    ar_in[:].rearrange("x b c -> (x b) c"), tp_sb[:]
                )
                if use_collective:
                    nc.gpsimd.collective_compute(
                        "AllGather",
                        mybir.AluOpType.bypass,
                        replica_groups=[list(range(NCORES))],
                        ins=[(ar_static if ar_indep else ar_in).opt()],
                        outs=[ar_out.opt()],
                    )
                else:
                    for g in range(NCORES):
                        nc.scalar.dma_start(ar_out[g : g + 1], ar_in[:])
                if warm:
                    warm_ps = psum.tile([1, 512], f32, tag="wm", bufs=1)
                    for i in range(warm):
                        nc.tensor.matmul(
                            warm_ps[:],
                            ones_red[0:JB, :],
                            wkt_sb[:, 0:512],
                            start=(i == 0),
                            stop=(i == warm - 1),
                        )

                state[rep] = (xc_sb, wqc_sb, t_full, ar_in, ar_out)

            def post_stage(rep):
                xc_sb, wqc_sb, t_full, ar_in, ar_out = state.pop(rep)
                if ar_unused:
                    for g in range(NCORES):
                        nc.scalar.dma_start(
                            t_full[g * B : (g + 1) * B, :],
                            ar_in[:].rearrange("x b c -> (x b) c"),
                        )
                else:
                    nc.scalar.dma_start(
                        t_full[:], ar_out[:].rearrange("g b c -> (g b) c")
                    )
                tT_sb = pool.tile([128, JT, B * NCORES], bf16, tag="tT", bufs=2)
                uT_sb = pool.tile([JB, B], bf16, tag="uT", bufs=2)
                out_sb = pool.tile([B, B * N], bf16, tag="os", bufs=2)

                for ck in range(JT):
                    tt_ps = psum.tile(
                        [128, B * NCORES], bf16, tag="tr", bufs=2, name=f"tt{ck}"
                    )
                    nc.tensor.transpose(
                        tt_ps[:],
                        t_full[:, ck * 128 : (ck + 1) * 128],
                        ident_g[:],
                    )
                    nc.vector.tensor_copy(tT_sb[:, ck, :], tt_ps[:])

                u_ps = psum.tile(
                    [JB, B * NCORES], f32, tag="u", bufs=1, name="u"
                )
                for ck in range(JT):
                    nc.tensor.matmul(
                        u_ps[:],
                        wqc_sb[:, ck, :],
                        tT_sb[:, ck, :],
                        start=(ck == 0),
                        stop=(ck == JT - 1),
                    )
                # sum the 8 gathered contributions per batch (strided views)
                u_red = pool.tile([JB, B], f32, tag="ur", bufs=2)
                for b in range(B):
                    nc.vector.reduce_sum(
                        u_red[:, b : b + 1],
                        u_ps[:].rearrange("j (g b) -> j g b", b=B)[:, :, b],
                        axis=AX,
                    )
                nc.scalar.mul(uT_sb[:], u_red[:], SCALE)

                for k in range(NSEG):
                    ps_ps = psum.tile(
                        [B, 512], f32, tag="ps", bufs=3, name=f"ps{k}"
                    )
                    nc.tensor.matmul(
                        ps_ps[:],
                        uT_sb[:],
                        xc_sb[:, k * 512 : (k + 1) * 512],
                        start=True,
                        stop=True,
                    )
                    if k % 3 == 0:
                        nc.scalar.copy(
                            out_sb[:, k * 512 : (k + 1) * 512], ps_ps[:]
                        )
                    else:
                        nc.vector.tensor_copy(
                            out_sb[:, k * 512 : (k + 1) * 512], ps_ps[:]
                        )
                for b in range(B):
                    nc.gpsimd.dma_start(
                        out_d[b : b + 1, :],
                        out_sb[b : b + 1, b * N : (b + 1) * N],
                    )

            for rep in range(repeats + lag):
                if rep < repeats:
                    pre_stage(rep)
                if rep >= lag:
                    post_stage(rep - lag)

    nc.compile()
    return nc


def _get_nc():
    global _compiled_nc
    if _compiled_nc is None:
        _compiled_nc = _build_and_compile()
    return _compiled_nc


def make_in_maps(X, W_qkv):
    import ml_dtypes

    bf = ml_dtypes.bfloat16
    X = np.ascontiguousarray(X, dtype=np.float32)
    W = np.ascontiguousarray(W_qkv, dtype=np.float32)
    assert X.shape == (B, N, C) and W.shape == (2 * C, C)
    XT = np.ascontiguousarray(X.transpose(2, 0, 1).reshape(C, B * N)).astype(bf)
    maps = []
    for i in range(NCORES):
        ci = slice(i * JB, (i + 1) * JB)
        wkt = np.ascontiguousarray(W[C : 2 * C, ci].T).astype(bf)    # [96, 768]
        wq = W[0:C, ci]                                              # [768, 96]
        wq_sw = np.ascontiguousarray(
            wq.reshape(JT, 128, JB).transpose(1, 0, 2).reshape(128, JT * JB)
        ).astype(bf)
        maps.append(
            {
                "x_in": np.ascontiguousarray(XT[ci]),
                "wkt_in": wkt,
                "wq_in": wq_sw,
            }
        )
    return maps


def assemble_out(results):
    acc = results[0]["scores_out"].astype(np.float32).copy()
    for i in range(1, NCORES):
        acc += results[i]["scores_out"]
    return acc


def kernel(X, W_qkv):
    from concourse import bass_utils

    nc = _get_nc()
    res = bass_utils.run_bass_kernel_spmd(
        nc, make_in_maps(X, W_qkv), core_ids=list(range(NCORES))
    )
    return assemble_out(res.results)


# revision 6
# speedup vs baseline: 1.1743x; 1.1743x over previous
"""Distributed TRN2 Bass kernel for nn_Att_scores (attention score double-sum).

Math: the reference computes
    qkv = X @ W_qkv.T ; q, k = split ; attn = (q @ k^T) * scale
    scores = attn.sum(heads).sum(keys)                          # [B, N]
Both sums commute with the matmuls, so no [N, N] attention matrix is needed:
    s[b]        = sum_n X[b, n, :]          (row-sum of X)       # [C]
    t[b]        = Wk @ s[b]                                      # [C]
    u[b]        = Wq^T @ t[b]                                    # [C]
    scores[b,n] = scale * X[b, n, :] . u[b]
i.e. one global row-sum, two 768x768 matvecs, one per-row dot.

Distribution: shard the CHANNEL axis across the 8 cores (96 channels each).
Core i holds X^T[ci, :, :] (its channels, ALL rows), Wk[:, ci]^T, Wq[:, ci]:
    s_i = sum_n X[b, n, ci]                 local (channel-complete, no AR)
    t_part_i = Wk[:, ci] @ s_i              local rank-96 partial of t
    AllGather(t_part, 3 KB/rank)            the ONLY collective
    u[ci] = Wq[:, ci]^T t                   the 8-way t-sum folds into the
                                            u-matmul free dim (f=16) + two
                                            tiny strided DVE reduces
    ps_i[b, n] = sum_{c in ci} X[b,n,c]u[c] partial scores over ALL rows
    host: scores = sum_i ps_i               unshard-by-sum in assemble_out
Per-core HBM traffic ~1.1 MB/iter (bf16): X slice 0.79 MB once, two 96-col
W blocks, one 3 KB gather, 16 KB store.  (The v1 baseline moved ~9.4 MB and
was pure-DMA-bound at ~20 us.)

Precision: host pre-casts the sharded inputs to bf16 (the kernel used SWDGE
cast-DMAs fp32->bf16 on the same values anyway, so device numerics are
unchanged); all accumulation is fp32 on PSUM; the AllGather payload is the
bf16 t-partials.  Measured absmax relative error vs the fp32 reference:
5.5e-3 (gate 2e-2).

Engine mapping / scheduling:
  * bulk loads ride the Sync HWDGE ring (2 X slabs + 2 W blocks, 2 KB/desc
    per partition; finer descriptors measurably LOSE to the sub-512B HBM
    penalty)
  * s_i: free-axis reduction split DVE reduce_sum (b=0) / ACT Copy+accum_out
    (b=1)
  * t_part: s-stationary TensorE matmul; drain split ACT/DVE
  * gather bounce + gathered-t load ride the ScalarE HWDGE ring; output
    stores are SWDGE cast-DMAs bf16->fp32 on the (otherwise idle) Pool queue
  * t^T: 6 TensorE transposes of the gathered [16, 768] + DVE copies;
    u: Wq-stationary matmul (f=16); SCALE folds into the uT PSUM drain
  * partial scores: 8 u-stationary [2, 512] matmuls, PSUM tag rotated 3-deep
    (bufs=3 was worth ~2 us: the drain chain was PSUM-buffer-limited);
    drains alternate ACT/DVE
  * engines execute in order, so the unrolled reps are emitted as a software
    pipeline with lag=5: block k = pre-stage(k) [loads, s, t_part, gather]
    + post-stage(k-5) [gathered load, t^T, u, scores, store].  Per-rep SBUF
    tiles rotate with bufs=lag+2.  Without this the collective's ~9 us
    round-trip latency lands on the TensorE critical path every rep.
  * AllGather beats AllReduce here (~2 us/iter measured): half the ring
    occupancy, and the sum it skips is nearly free on-device.
"""

import numpy as np

B = 2
N = 2048
C = 768
H = 12
HD = C // H
SCALE = float(HD) ** -0.5
NCORES = 8
JB = C // NCORES          # 96 channels per core
JT = C // 128             # 6 chunks of 128 rows of Wq
NSEG = (B * N) // 512     # 8 output segments of 512 columns

_compiled_nc = None


def _build_and_compile(use_collective=True, repeats=1, warm=0, lag=5, ar_unused=False, ar_indep=False, nslab=2):
    import concourse.bass as bass  # noqa: F401
    import concourse.bacc as bacc
    import concourse.tile as tile
    import concourse.mybir as mybir
    from concourse import masks

    f32 = mybir.dt.float32
    bf16 = mybir.dt.bfloat16
    add = mybir.AluOpType.add
    copy_fn = mybir.ActivationFunctionType.Copy
    AX = mybir.AxisListType.X

    nc = bacc.Bacc(
        "TRN2",
        target_bir_lowering=False,
        debug=False,
        num_devices=NCORES,
    )

    x_d = nc.dram_tensor("x_in", [JB, B * N], bf16, kind="ExternalInput")
    wkt_d = nc.dram_tensor("wkt_in", [JB, C], bf16, kind="ExternalInput")
    wq_d = nc.dram_tensor("wq_in", [128, JT * JB], bf16, kind="ExternalInput")
    out_d = nc.dram_tensor("scores_out", [B, N], f32, kind="ExternalOutput")

    SLICES = ((0, 512), (512, 256))
    lag = min(lag, max(repeats - 1, 0))
    sbufs = lag + 2

    with tile.TileContext(nc) as tc:
        with (
            tc.tile_pool(name="sbuf", bufs=1) as pool,
            tc.tile_pool(name="psum", bufs=1, space="PSUM") as psum,
            tc.tile_pool(name="dram", bufs=1, space="DRAM") as dram,
        ):
            ones_red = pool.tile([128, 1], bf16)
            ident_g = pool.tile([2 * NCORES, 2 * NCORES], bf16)
            nc.gpsimd.memset(ones_red[:], 1.0)
            masks.make_identity(nc, ident_g[:])
            ar_static = dram.tile([1, B, C], bf16, name="ar_static")
            if ar_indep:
                zz = pool.tile([1, B * C], bf16, name="zz")
                nc.gpsimd.memset(zz[:], 0.0)
                nc.scalar.dma_start(
                    ar_static[:].rearrange("x b c -> x (b c)"), zz[:]
                )

            state = {}

            def pre_stage(rep):
                xc_sb = pool.tile([JB, B * N], bf16, tag="xc", bufs=sbufs)
                scratch = pool.tile([JB, 2048], bf16, tag="scr", bufs=2)
                wkt_sb = pool.tile([JB, C], bf16, tag="wkt", bufs=sbufs)
                wqc_sb = pool.tile([128, JT, JB], bf16, tag="wqc", bufs=sbufs)
                s_f = pool.tile([JB, 4], f32, tag="sf", bufs=2)
                s_bf = pool.tile([JB, B], bf16, tag="sbf", bufs=2)
                tp_sb = pool.tile([B, C], bf16, tag="tp", bufs=2)
                t_full = pool.tile([B * NCORES, C], bf16, tag="tf", bufs=sbufs)

                slab = (B * N) // nslab
                for q in range(nslab):
                    nc.sync.dma_start(
                        xc_sb[:, q * slab : (q + 1) * slab],
                        x_d[:, q * slab : (q + 1) * slab],
                    )
                nc.sync.dma_start(wkt_sb[:], wkt_d.ap())
                nc.sync.dma_start(
                    wqc_sb[:], wq_d.ap().rearrange("p (t f) -> p t f", t=JT)
                )

                nc.vector.reduce_sum(s_f[:, 0:1], xc_sb[:, 0:2048], axis=AX)
                nc.scalar.activation(
                    scratch[:, 0:2048], xc_sb[:, 2048:4096],
                    copy_fn, accum_out=s_f[:, 2:3],
                )
                nc.vector.tensor_copy(s_bf[:, 0:1], s_f[:, 0:1])
                nc.vector.tensor_copy(s_bf[:, 1:2], s_f[:, 2:3])

                tp_ps = psum.tile([B, 1024], f32, tag="tp", bufs=1, name="tp")
                for lo, nsz in SLICES:
                    nc.tensor.matmul(
                        tp_ps[:, lo : lo + nsz],
                        s_bf[:],
                        wkt_sb[:, lo : lo + nsz],
                        start=True,
                        stop=True,
                    )
                nc.scalar.copy(tp_sb[:, 0:512], tp_ps[:, 0:512])
                nc.vector.tensor_copy(tp_sb[:, 512:C], tp_ps[:, 512:C])

                ar_in = dram.tile([1, B, C], bf16, name=f"ar_in{rep}")
                ar_out = dram.tile(
                    [NCORES, B, C], bf16, addr_space="Shared", name=f"ar_out{rep}"
                )
                nc.scalar.dma_start(
                    ar_in[:].rearrange("x b c -> (x b) c"), tp_sb[:]
                )
                if use_collective:
                    nc.gpsimd.collective_compute(
                        "AllGather",
                        mybir.AluOpType.bypass,
                        replica_groups=[list(range(NCORES))],
                        ins=[(ar_static if ar_indep else ar_in).opt()],
                        outs=[ar_out.opt()],
                    )
                else:
                    for g in range(NCORES):
                        nc.scalar.dma_start(ar_out[g : g + 1], ar_in[:])
                if warm:
                    warm_ps = psum.tile([1, 512], f32, tag="wm", bufs=1)
                    for i in range(warm):
                        nc.tensor.matmul(
                            warm_ps[:],
                            ones_red[0:JB, :],
                            wkt_sb[:, 0:512],
                            start=(i == 0),
                            stop=(i == warm - 1),
                        )

                state[rep] = (xc_sb, wqc_sb, t_full, ar_in, ar_out)

            def post_stage(rep):
                xc_sb, wqc_sb, t_full, ar_in, ar_out = state.pop(rep)
                if ar_unused:
                    for g in range(NCORES):
                        nc.scalar.dma_start(
                            t_full[g * B : (g + 1) * B, :],
                            ar_in[:].rearrange("x b c -> (x b) c"),
                        )
                else:
                    nc.scalar.dma_start(
                        t_full[:], ar_out[:].rearrange("g b c -> (g b) c")
                    )
                tT_sb = pool.tile([128, JT, B * NCORES], bf16, tag="tT", bufs=2)
                uT_sb = pool.tile([JB, B], bf16, tag="uT", bufs=2)
                out_sb = pool.tile([B, B * N], bf16, tag="os", bufs=2)

                for ck in range(JT):
                    tt_ps = psum.tile(
                        [128, B * NCORES], bf16, tag="tr", bufs=2, name=f"tt{ck}"
                    )
                    nc.tensor.transpose(
                        tt_ps[:],
                        t_full[:, ck * 128 : (ck + 1) * 128],
                        ident_g[:],
                    )
                    nc.vector.tensor_copy(tT_sb[:, ck, :], tt_ps[:])

                u_ps = psum.tile(
                    [JB, B * NCORES], f32, tag="u", bufs=1, name="u"
                )
                for ck in range(JT):
                    nc.tensor.matmul(
                        u_ps[:],
                        wqc_sb[:, ck, :],
                        tT_sb[:, ck, :],
                        start=(ck == 0),
                        stop=(ck == JT - 1),
                    )
                # sum the 8 gathered contributions per batch (strided views)
                u_red = pool.tile([JB, B], f32, tag="ur", bufs=2)
                for b in range(B):
                    nc.vector.reduce_sum(
                        u_red[:, b : b + 1],
                        u_ps[:].rearrange("j (g b) -> j g b", b=B)[:, :, b],
                        axis=AX,
                    )
                nc.scalar.mul(uT_sb[:], u_red[:], SCALE)

                for k in range(NSEG):
                    ps_ps = psum.tile(
                        [B, 512], f32, tag="ps", bufs=3, name=f"ps{k}"
                    )
                    nc.tensor.matmul(
                        ps_ps[:],
                        uT_sb[:],
                        xc_sb[:, k * 512 : (k + 1) * 512],
                        start=True,
                        stop=True,
                    )
                    if k % 2 == 0:
                        nc.scalar.copy(
                            out_sb[:, k * 512 : (k + 1) * 512], ps_ps[:]
                        )
                    else:
                        nc.vector.tensor_copy(
                            out_sb[:, k * 512 : (k + 1) * 512], ps_ps[:]
                        )
                for b in range(B):
                    nc.gpsimd.dma_start(
                        out_d[b : b + 1, :],
                        out_sb[b : b + 1, b * N : (b + 1) * N],
                    )

            for rep in range(repeats + lag):
                if rep < repeats:
                    pre_stage(rep)
                if rep >= lag:
                    post_stage(rep - lag)

    nc.compile()
    return nc


def _get_nc():
    global _compiled_nc
    if _compiled_nc is None:
        _compiled_nc = _build_and_compile()
    return _compiled_nc


def make_in_maps(X, W_qkv):
    import ml_dtypes

    bf = ml_dtypes.bfloat16
    X = np.ascontiguousarray(X, dtype=np.float32)
    W = np.ascontiguousarray(W_qkv, dtype=np.float32)
    assert X.shape == (B, N, C) and W.shape == (2 * C, C)
    XT = np.ascontiguousarray(X.transpose(2, 0, 1).reshape(C, B * N)).astype(bf)
    maps = []
    for i in range(NCORES):
        ci = slice(i * JB, (i + 1) * JB)
        wkt = np.ascontiguousarray(W[C : 2 * C, ci].T).astype(bf)    # [96, 768]
        wq = W[0:C, ci]                                              # [768, 96]
        wq_sw = np.ascontiguousarray(
            wq.reshape(JT, 128, JB).transpose(1, 0, 2).reshape(128, JT * JB)
        ).astype(bf)
        maps.append(
            {
                "x_in": np.ascontiguousarray(XT[ci]),
                "wkt_in": wkt,
                "wq_in": wq_sw,
            }
        )
    return maps


def assemble_out(results):
    acc = results[0]["scores_out"].astype(np.float32).copy()
    for i in range(1, NCORES):
        acc += results[i]["scores_out"]
    return acc


def kernel(X, W_qkv):
    from concourse import bass_utils

    nc = _get_nc()
    res = bass_utils.run_bass_kernel_spmd(
        nc, make_in_maps(X, W_qkv), core_ids=list(range(NCORES))
    )
    return assemble_out(res.results)


# revision 7
# speedup vs baseline: 2.4452x; 2.0822x over previous
"""Distributed TRN2 Bass kernel for nn_Att_scores (attention score double-sum).

Math: the reference computes
    qkv = X @ W_qkv.T ; q, k = split ; attn = (q @ k^T) * scale
    scores = attn.sum(heads).sum(keys)                          # [B, N]
Both sums commute with the matmuls, so no [N, N] attention matrix is needed:
    s[b]        = sum_n X[b, n, :]          (row-sum of X)       # [C]
    t[b]        = Wk @ s[b]                                      # [C]
    u[b]        = Wq^T @ t[b]                                    # [C]
    scores[b,n] = scale * X[b, n, :] . u[b]
i.e. one global row-sum, two 768x768 matvecs, one per-row dot.

Distribution: shard the CHANNEL axis across the 8 cores (96 channels each).
Core i holds X^T[ci, :, :] (its channels, ALL rows), Wk[:, ci]^T, Wq[:, ci]:
    s_i = sum_n X[b, n, ci]                 local (channel-complete, no AR)
    t_part_i = Wk[:, ci] @ s_i              local rank-96 partial of t
    AllGather(t_part, 3 KB/rank)            the ONLY collective
    u[ci] = Wq[:, ci]^T t                   the 8-way t-sum folds into the
                                            u-matmul free dim (f=16) + two
                                            tiny strided DVE reduces
    ps_i[b, n] = sum_{c in ci} X[b,n,c]u[c] partial scores over ALL rows
    host: scores = sum_i ps_i               unshard-by-sum in assemble_out
Per-core HBM traffic ~1.1 MB/iter (bf16): X slice 0.79 MB once, two 96-col
W blocks, one 3 KB gather, 16 KB store.  (The v1 baseline moved ~9.4 MB and
was pure-DMA-bound at ~20 us.)

Precision: host pre-casts the sharded inputs to bf16 (the kernel used SWDGE
cast-DMAs fp32->bf16 on the same values anyway, so device numerics are
unchanged); all accumulation is fp32 on PSUM; the AllGather payload is the
bf16 t-partials.  Measured absmax relative error vs the fp32 reference:
5.5e-3 (gate 2e-2).

Engine mapping / scheduling:
  * bulk loads ride the Sync HWDGE ring (2 X slabs + 2 W blocks, 2 KB/desc
    per partition; finer descriptors measurably LOSE to the sub-512B HBM
    penalty)
  * s_i: free-axis reduction split DVE reduce_sum (b=0) / ACT Copy+accum_out
    (b=1)
  * t_part: s-stationary TensorE matmul; drain split ACT/DVE
  * gather bounce + gathered-t load ride the ScalarE HWDGE ring; output
    stores are SWDGE cast-DMAs bf16->fp32 on the (otherwise idle) Pool queue
  * t^T: 6 TensorE transposes of the gathered [16, 768] + DVE copies;
    u: Wq-stationary matmul (f=16); SCALE folds into the uT PSUM drain
  * partial scores: 8 u-stationary [2, 512] matmuls, PSUM tag rotated 3-deep
    (bufs=3 was worth ~2 us: the drain chain was PSUM-buffer-limited);
    drains alternate ACT/DVE
  * engines execute in order, so the unrolled reps are emitted as a software
    pipeline with lag=5: block k = pre-stage(k) [loads, s, t_part, gather]
    + post-stage(k-5) [gathered load, t^T, u, scores, store].  Per-rep SBUF
    tiles rotate with bufs=lag+2.  Without this the collective's ~9 us
    round-trip latency lands on the TensorE critical path every rep.
  * AllGather beats AllReduce here (~2 us/iter measured): half the ring
    occupancy, and the sum it skips is nearly free on-device.
"""

import numpy as np

B = 2
N = 2048
C = 768
H = 12
HD = C // H
SCALE = float(HD) ** -0.5
NCORES = 8
JB = C // NCORES          # 96 channels per core
JT = C // 128             # 6 chunks of 128 rows of Wq
NSEG = (B * N) // 512     # 8 output segments of 512 columns

_compiled_nc = None


def _build_and_compile(use_collective=True, repeats=1, warm=0, lag=5, ar_unused=False, ar_indep=False, nslab=1):
    import concourse.bass as bass  # noqa: F401
    import concourse.bacc as bacc
    import concourse.tile as tile
    import concourse.mybir as mybir
    from concourse import masks

    f32 = mybir.dt.float32
    bf16 = mybir.dt.bfloat16
    add = mybir.AluOpType.add
    copy_fn = mybir.ActivationFunctionType.Copy
    AX = mybir.AxisListType.X

    nc = bacc.Bacc(
        "TRN2",
        target_bir_lowering=False,
        debug=False,
        num_devices=NCORES,
    )

    x_d = nc.dram_tensor("x_in", [JB, B * N], bf16, kind="ExternalInput")
    wkt_d = nc.dram_tensor("wkt_in", [JB, C], bf16, kind="ExternalInput")
    wq_d = nc.dram_tensor("wq_in", [128, JT * JB], bf16, kind="ExternalInput")
    out_d = nc.dram_tensor("scores_out", [B, N], bf16, kind="ExternalOutput")

    SLICES = ((0, 512), (512, 256))
    lag = min(lag, max(repeats - 1, 0))
    sbufs = lag + 2

    with tile.TileContext(nc) as tc:
        with (
            tc.tile_pool(name="sbuf", bufs=1) as pool,
            tc.tile_pool(name="psum", bufs=1, space="PSUM") as psum,
            tc.tile_pool(name="dram", bufs=1, space="DRAM") as dram,
        ):
            ones_red = pool.tile([128, 1], bf16)
            ident_g = pool.tile([2 * NCORES, 2 * NCORES], bf16)
            nc.gpsimd.memset(ones_red[:], 1.0)
            masks.make_identity(nc, ident_g[:])
            ar_static = dram.tile([1, B, C], bf16, name="ar_static")
            if ar_indep:
                zz = pool.tile([1, B * C], bf16, name="zz")
                nc.gpsimd.memset(zz[:], 0.0)
                nc.scalar.dma_start(
                    ar_static[:].rearrange("x b c -> x (b c)"), zz[:]
                )

            state = {}

            def pre_stage(rep):
                xc_sb = pool.tile([JB, B * N], bf16, tag="xc", bufs=sbufs)
                scratch = pool.tile([JB, 2048], bf16, tag="scr", bufs=2)
                wkt_sb = pool.tile([JB, C], bf16, tag="wkt", bufs=sbufs)
                wqc_sb = pool.tile([128, JT, JB], bf16, tag="wqc", bufs=sbufs)
                s_f = pool.tile([JB, 4], f32, tag="sf", bufs=2)
                s_bf = pool.tile([JB, B], bf16, tag="sbf", bufs=2)
                tp_sb = pool.tile([B, C], bf16, tag="tp", bufs=2)
                t_full = pool.tile([B * NCORES, C], bf16, tag="tf", bufs=sbufs)

                slab = (B * N) // nslab
                for q in range(nslab):
                    nc.sync.dma_start(
                        xc_sb[:, q * slab : (q + 1) * slab],
                        x_d[:, q * slab : (q + 1) * slab],
                    )
                nc.sync.dma_start(wkt_sb[:], wkt_d.ap())
                nc.sync.dma_start(
                    wqc_sb[:], wq_d.ap().rearrange("p (t f) -> p t f", t=JT)
                )

                tre = pool.tile([JB, 1536], bf16, tag="tre", bufs=2)
                nc.vector.tensor_add(
                    tre[:, 0:1024], xc_sb[:, 0:1024], xc_sb[:, 1024:2048]
                )
                nc.vector.tensor_add(
                    tre[:, 1024:1536], tre[:, 0:512], tre[:, 512:1024]
                )
                nc.vector.reduce_sum(s_f[:, 0:1], tre[:, 1024:1536], axis=AX)
                nc.scalar.activation(
                    scratch[:, 0:2048], xc_sb[:, 2048:4096],
                    copy_fn, accum_out=s_f[:, 2:3],
                )
                nc.vector.tensor_copy(s_bf[:, 0:1], s_f[:, 0:1])
                nc.vector.tensor_copy(s_bf[:, 1:2], s_f[:, 2:3])

                tp_ps = psum.tile([B, 1024], f32, tag="tp", bufs=1, name="tp")
                for lo, nsz in SLICES:
                    nc.tensor.matmul(
                        tp_ps[:, lo : lo + nsz],
                        s_bf[:],
                        wkt_sb[:, lo : lo + nsz],
                        start=True,
                        stop=True,
                    )
                nc.scalar.copy(tp_sb[:, 0:512], tp_ps[:, 0:512])
                nc.vector.tensor_copy(tp_sb[:, 512:C], tp_ps[:, 512:C])

                ar_in = dram.tile([1, B, C], bf16, name=f"ar_in{rep}")
                ar_out = dram.tile(
                    [NCORES, B, C], bf16, addr_space="Shared", name=f"ar_out{rep}"
                )
                nc.scalar.dma_start(
                    ar_in[:].rearrange("x b c -> (x b) c"), tp_sb[:]
                )
                if use_collective:
                    nc.gpsimd.collective_compute(
                        "AllGather",
                        mybir.AluOpType.bypass,
                        replica_groups=[list(range(NCORES))],
                        ins=[(ar_static if ar_indep else ar_in).opt()],
                        outs=[ar_out.opt()],
                    )
                else:
                    for g in range(NCORES):
                        nc.scalar.dma_start(ar_out[g : g + 1], ar_in[:])
                if warm:
                    warm_ps = psum.tile([1, 512], f32, tag="wm", bufs=1)
                    for i in range(warm):
                        nc.tensor.matmul(
                            warm_ps[:],
                            ones_red[0:JB, :],
                            wkt_sb[:, 0:512],
                            start=(i == 0),
                            stop=(i == warm - 1),
                        )

                state[rep] = (xc_sb, wqc_sb, t_full, ar_in, ar_out)

            def post_stage(rep):
                xc_sb, wqc_sb, t_full, ar_in, ar_out = state.pop(rep)
                if ar_unused:
                    for g in range(NCORES):
                        nc.scalar.dma_start(
                            t_full[g * B : (g + 1) * B, :],
                            ar_in[:].rearrange("x b c -> (x b) c"),
                        )
                else:
                    nc.scalar.dma_start(
                        t_full[:], ar_out[:].rearrange("g b c -> (g b) c")
                    )
                tT_sb = pool.tile([128, JT, B * NCORES], bf16, tag="tT", bufs=2)
                uT_sb = pool.tile([JB, B], bf16, tag="uT", bufs=2)
                out_sb = pool.tile([B, B * N], bf16, tag="os", bufs=2)

                for ck in range(JT):
                    tt_ps = psum.tile(
                        [128, B * NCORES], bf16, tag="tr", bufs=2, name=f"tt{ck}"
                    )
                    nc.tensor.transpose(
                        tt_ps[:],
                        t_full[:, ck * 128 : (ck + 1) * 128],
                        ident_g[:],
                    )
                    nc.vector.tensor_copy(tT_sb[:, ck, :], tt_ps[:])

                u_ps = psum.tile(
                    [JB, B * NCORES], f32, tag="u", bufs=1, name="u"
                )
                for ck in range(JT):
                    nc.tensor.matmul(
                        u_ps[:],
                        wqc_sb[:, ck, :],
                        tT_sb[:, ck, :],
                        start=(ck == 0),
                        stop=(ck == JT - 1),
                    )
                # sum the 8 gathered contributions per batch (strided views)
                u_red = pool.tile([JB, B], f32, tag="ur", bufs=2)
                for b in range(B):
                    nc.vector.reduce_sum(
                        u_red[:, b : b + 1],
                        u_ps[:].rearrange("j (g b) -> j g b", b=B)[:, :, b],
                        axis=AX,
                    )
                nc.scalar.mul(uT_sb[:], u_red[:], SCALE)

                for k in range(NSEG):
                    ps_ps = psum.tile(
                        [B, 512], f32, tag="ps", bufs=3, name=f"ps{k}"
                    )
                    nc.tensor.matmul(
                        ps_ps[:],
                        uT_sb[:],
                        xc_sb[:, k * 512 : (k + 1) * 512],
                        start=True,
                        stop=True,
                    )
                    if k % 2 == 0:
                        nc.scalar.copy(
                            out_sb[:, k * 512 : (k + 1) * 512], ps_ps[:]
                        )
                    else:
                        nc.vector.tensor_copy(
                            out_sb[:, k * 512 : (k + 1) * 512], ps_ps[:]
                        )
                for b in range(B):
                    nc.sync.dma_start(
                        out_d[b : b + 1, :],
                        out_sb[b : b + 1, b * N : (b + 1) * N],
                    )

            for rep in range(repeats + lag):
                if rep < repeats:
                    pre_stage(rep)
                if rep >= lag:
                    post_stage(rep - lag)

    nc.compile()
    return nc


def _get_nc():
    global _compiled_nc
    if _compiled_nc is None:
        _compiled_nc = _build_and_compile()
    return _compiled_nc


def make_in_maps(X, W_qkv):
    import ml_dtypes

    bf = ml_dtypes.bfloat16
    X = np.ascontiguousarray(X, dtype=np.float32)
    W = np.ascontiguousarray(W_qkv, dtype=np.float32)
    assert X.shape == (B, N, C) and W.shape == (2 * C, C)
    XT = np.ascontiguousarray(X.transpose(2, 0, 1).reshape(C, B * N)).astype(bf)
    maps = []
    for i in range(NCORES):
        ci = slice(i * JB, (i + 1) * JB)
        wkt = np.ascontiguousarray(W[C : 2 * C, ci].T).astype(bf)    # [96, 768]
        wq = W[0:C, ci]                                              # [768, 96]
        wq_sw = np.ascontiguousarray(
            wq.reshape(JT, 128, JB).transpose(1, 0, 2).reshape(128, JT * JB)
        ).astype(bf)
        maps.append(
            {
                "x_in": np.ascontiguousarray(XT[ci]),
                "wkt_in": wkt,
                "wq_in": wq_sw,
            }
        )
    return maps


def assemble_out(results):
    acc = results[0]["scores_out"].astype(np.float32).copy()
    for i in range(1, NCORES):
        acc += results[i]["scores_out"]
    return acc


def kernel(X, W_qkv):
    from concourse import bass_utils

    nc = _get_nc()
    res = bass_utils.run_bass_kernel_spmd(
        nc, make_in_maps(X, W_qkv), core_ids=list(range(NCORES))
    )
    return assemble_out(res.results)


# revision 8
# speedup vs baseline: 3.3925x; 1.3874x over previous
"""Distributed TRN2 Bass kernel for nn_Att_scores (attention score double-sum).

Math: the reference computes
    qkv = X @ W_qkv.T ; q, k = split ; attn = (q @ k^T) * scale
    scores = attn.sum(heads).sum(keys)                          # [B, N]
Both sums commute with the matmuls, so no [N, N] attention matrix is needed:
    s[b]        = sum_n X[b, n, :]          (row-sum of X)       # [C]
    t[b]        = Wk @ s[b]                                      # [C]
    u[b]        = Wq^T @ t[b]                                    # [C]
    scores[b,n] = scale * X[b, n, :] . u[b]
i.e. one global row-sum, two 768x768 matvecs, one per-row dot.

Distribution: shard the CHANNEL axis across the 8 cores (96 channels each).
Core i holds X^T[ci, :, :] (its channels, ALL rows), Wk[:, ci]^T, Wq[:, ci]:
    s_i = sum_n X[b, n, ci]                 local (channel-complete, no AR)
    t_part_i = Wk[:, ci] @ s_i              local rank-96 partial of t
    AllGather(t_part, 3 KB/rank)            the ONLY collective
    u[ci] = Wq[:, ci]^T t                   the 8-way t-sum folds into the
                                            u-matmul free dim (f=16) + two
                                            tiny strided DVE reduces
    ps_i[b, n] = sum_{c in ci} X[b,n,c]u[c] partial scores over ALL rows
    host: scores = sum_i ps_i               unshard-by-sum in assemble_out
Per-core HBM traffic ~1.1 MB/iter (bf16): X slice 0.79 MB once, two 96-col
W blocks, one 3 KB gather, 8 KB store.  (The v1 baseline moved ~9.4 MB and
was pure-DMA-bound at ~20 us.)

Precision: host pre-casts the sharded inputs to bf16 (the kernel used SWDGE
cast-DMAs fp32->bf16 on the same values anyway, so device numerics are
unchanged); accumulation is fp32 on PSUM; the AllGather payload and the
partial-score output are bf16 (assemble_out upcasts).  Measured absmax
relative error vs the fp32 reference: 5.5e-3 (gate 2e-2).

Engine mapping / scheduling (hard-won points in CAPS):
  * bulk loads ride the Sync HWDGE ring, one slab per tensor, 2 KB/desc per
    partition (finer descriptors LOSE to the sub-512B HBM penalty)
  * s_i: b=0 on DVE as a 2-level pairwise-add tree (tensor_add is 2x-rate
    for bf16 where tensor_reduce is 1x-capped) + short reduce; b=1 on ACT
    via activation Copy+accum_out
  * t_part: s-stationary TensorE matmul; drain split ACT/DVE
  * gather bounce + gathered-t load ride the ScalarE HWDGE ring
  * output stores are PLAIN bf16 DMAs on the Sync ring.  NEVER put per-rep
    SWDGE DMAs on the Pool queue: Pool also issues the collectives, and two
    ~2 us descriptor generations per rep were serializing the collective
    lane (~3 us/iter penalty)
  * t^T: 6 TensorE transposes of the gathered [16, 768] + DVE copies;
    u: Wq-stationary matmul (f=16); SCALE folds into the uT PSUM drain
  * partial scores: 8 u-stationary [2, 512] matmuls, PSUM tag rotated 3-deep
    (bufs=3 was worth ~2 us: the drain chain was PSUM-buffer-limited);
    drains alternate ACT/DVE
  * engines execute in order, so the unrolled reps are emitted as a software
    pipeline with lag=5: block k = pre-stage(k) [loads, s, t_part, gather]
    + post-stage(k-5) [gathered load, t^T, u, scores, store].  Per-rep SBUF
    tiles rotate with bufs=lag+2.  Without this the collective round-trip
    (~9 us) lands on the TensorE critical path every rep.
  * AllGather beats AllReduce here (~2 us/iter): half the ring occupancy,
    and the skipped sum is nearly free on-device.  Groups of 4 are not
    supported by the collectives stack ("shared output needs >4 cores").
"""

import numpy as np

B = 2
N = 2048
C = 768
H = 12
HD = C // H
SCALE = float(HD) ** -0.5
NCORES = 8
JB = C // NCORES          # 96 channels per core
JT = C // 128             # 6 chunks of 128 rows of Wq
NSEG = (B * N) // 512     # 8 output segments of 512 columns

_compiled_nc = None


def _build_and_compile(use_collective=True, repeats=1, warm=0, lag=5, ar_unused=False, ar_indep=False, nslab=1):
    import concourse.bass as bass  # noqa: F401
    import concourse.bacc as bacc
    import concourse.tile as tile
    import concourse.mybir as mybir
    from concourse import masks

    f32 = mybir.dt.float32
    bf16 = mybir.dt.bfloat16
    add = mybir.AluOpType.add
    copy_fn = mybir.ActivationFunctionType.Copy
    AX = mybir.AxisListType.X

    nc = bacc.Bacc(
        "TRN2",
        target_bir_lowering=False,
        debug=False,
        num_devices=NCORES,
    )

    x_d = nc.dram_tensor("x_in", [JB, B * N], bf16, kind="ExternalInput")
    wkt_d = nc.dram_tensor("wkt_in", [JB, C], bf16, kind="ExternalInput")
    wq_d = nc.dram_tensor("wq_in", [128, JT * JB], bf16, kind="ExternalInput")
    out_d = nc.dram_tensor("scores_out", [B, N], bf16, kind="ExternalOutput")

    SLICES = ((0, 512), (512, 256))
    lag = min(lag, max(repeats - 1, 0))
    sbufs = lag + 2

    with tile.TileContext(nc) as tc:
        with (
            tc.tile_pool(name="sbuf", bufs=1) as pool,
            tc.tile_pool(name="psum", bufs=1, space="PSUM") as psum,
            tc.tile_pool(name="dram", bufs=1, space="DRAM") as dram,
        ):
            ones_red = pool.tile([128, 1], bf16)
            ident_g = pool.tile([2 * NCORES, 2 * NCORES], bf16)
            nc.gpsimd.memset(ones_red[:], 1.0)
            masks.make_identity(nc, ident_g[:])
            ar_static = dram.tile([1, B, C], bf16, name="ar_static")
            if ar_indep:
                zz = pool.tile([1, B * C], bf16, name="zz")
                nc.gpsimd.memset(zz[:], 0.0)
                nc.scalar.dma_start(
                    ar_static[:].rearrange("x b c -> x (b c)"), zz[:]
                )

            state = {}

            def pre_stage(rep):
                xc_sb = pool.tile([JB, B * N], bf16, tag="xc", bufs=sbufs)
                scratch = pool.tile([JB, 2048], bf16, tag="scr", bufs=2)
                wkt_sb = pool.tile([JB, C], bf16, tag="wkt", bufs=sbufs)
                wqc_sb = pool.tile([128, JT, JB], bf16, tag="wqc", bufs=sbufs)
                s_f = pool.tile([JB, 4], f32, tag="sf", bufs=2)
                s_bf = pool.tile([JB, B], bf16, tag="sbf", bufs=2)
                tp_sb = pool.tile([B, C], bf16, tag="tp", bufs=2)
                t_full = pool.tile([B * NCORES, C], bf16, tag="tf", bufs=sbufs)

                slab = (B * N) // nslab
                for q in range(nslab):
                    nc.sync.dma_start(
                        xc_sb[:, q * slab : (q + 1) * slab],
                        x_d[:, q * slab : (q + 1) * slab],
                    )
                nc.sync.dma_start(wkt_sb[:], wkt_d.ap())
                nc.sync.dma_start(
                    wqc_sb[:], wq_d.ap().rearrange("p (t f) -> p t f", t=JT)
                )

                tre = pool.tile([JB, 1536], bf16, tag="tre", bufs=2)
                nc.vector.tensor_add(
                    tre[:, 0:1024], xc_sb[:, 0:1024], xc_sb[:, 1024:2048]
                )
                nc.vector.tensor_add(
                    tre[:, 1024:1536], tre[:, 0:512], tre[:, 512:1024]
                )
                nc.vector.reduce_sum(s_f[:, 0:1], tre[:, 1024:1536], axis=AX)
                nc.scalar.activation(
                    scratch[:, 0:2048], xc_sb[:, 2048:4096],
                    copy_fn, accum_out=s_f[:, 2:3],
                )
                nc.vector.tensor_copy(s_bf[:, 0:1], s_f[:, 0:1])
                nc.vector.tensor_copy(s_bf[:, 1:2], s_f[:, 2:3])

                tp_ps = psum.tile([B, 1024], f32, tag="tp", bufs=1, name="tp")
                for lo, nsz in SLICES:
                    nc.tensor.matmul(
                        tp_ps[:, lo : lo + nsz],
                        s_bf[:],
                        wkt_sb[:, lo : lo + nsz],
                        start=True,
                        stop=True,
                    )
                nc.scalar.copy(tp_sb[:, 0:512], tp_ps[:, 0:512])
                nc.vector.tensor_copy(tp_sb[:, 512:C], tp_ps[:, 512:C])

                ar_in = dram.tile([1, B, C], bf16, name=f"ar_in{rep}")
                ar_out = dram.tile(
                    [NCORES, B, C], bf16, addr_space="Shared", name=f"ar_out{rep}"
                )
                nc.scalar.dma_start(
                    ar_in[:].rearrange("x b c -> (x b) c"), tp_sb[:]
                )
                if use_collective:
                    nc.gpsimd.collective_compute(
                        "AllGather",
                        mybir.AluOpType.bypass,
                        replica_groups=[list(range(NCORES))],
                        ins=[(ar_static if ar_indep else ar_in).opt()],
                        outs=[ar_out.opt()],
                    )
                else:
                    for g in range(NCORES):
                        nc.scalar.dma_start(ar_out[g : g + 1], ar_in[:])
                if warm:
                    warm_ps = psum.tile([1, 512], f32, tag="wm", bufs=1)
                    for i in range(warm):
                        nc.tensor.matmul(
                            warm_ps[:],
                            ones_red[0:JB, :],
                            wkt_sb[:, 0:512],
                            start=(i == 0),
                            stop=(i == warm - 1),
                        )

                state[rep] = (xc_sb, wqc_sb, t_full, ar_in, ar_out)

            def post_stage(rep):
                xc_sb, wqc_sb, t_full, ar_in, ar_out = state.pop(rep)
                if ar_unused:
                    for g in range(NCORES):
                        nc.scalar.dma_start(
                            t_full[g * B : (g + 1) * B, :],
                            ar_in[:].rearrange("x b c -> (x b) c"),
                        )
                else:
                    nc.scalar.dma_start(
                        t_full[:], ar_out[:].rearrange("g b c -> (g b) c")
                    )
                tT_sb = pool.tile([128, JT, B * NCORES], bf16, tag="tT", bufs=2)
                uT_sb = pool.tile([JB, B], bf16, tag="uT", bufs=2)
                out_sb = pool.tile([B, B * N], bf16, tag="os", bufs=2)

                for ck in range(JT):
                    tt_ps = psum.tile(
                        [128, B * NCORES], bf16, tag="tr", bufs=2, name=f"tt{ck}"
                    )
                    nc.tensor.transpose(
                        tt_ps[:],
                        t_full[:, ck * 128 : (ck + 1) * 128],
                        ident_g[:],
                    )
                    nc.vector.tensor_copy(tT_sb[:, ck, :], tt_ps[:])

                u_ps = psum.tile(
                    [JB, B * NCORES], f32, tag="u", bufs=1, name="u"
                )
                for ck in range(JT):
                    nc.tensor.matmul(
                        u_ps[:],
                        wqc_sb[:, ck, :],
                        tT_sb[:, ck, :],
                        start=(ck == 0),
                        stop=(ck == JT - 1),
                    )
                # sum the 8 gathered contributions per batch (strided views)
                u_red = pool.tile([JB, B], f32, tag="ur", bufs=2)
                for b in range(B):
                    nc.vector.reduce_sum(
                        u_red[:, b : b + 1],
                        u_ps[:].rearrange("j (g b) -> j g b", b=B)[:, :, b],
                        axis=AX,
                    )
                nc.scalar.mul(uT_sb[:], u_red[:], SCALE)

                for k in range(NSEG):
                    ps_ps = psum.tile(
                        [B, 512], f32, tag="ps", bufs=3, name=f"ps{k}"
                    )
                    nc.tensor.matmul(
                        ps_ps[:],
                        uT_sb[:],
                        xc_sb[:, k * 512 : (k + 1) * 512],
                        start=True,
                        stop=True,
                    )
                    if k % 2 == 0:
                        nc.scalar.copy(
                            out_sb[:, k * 512 : (k + 1) * 512], ps_ps[:]
                        )
                    else:
                        nc.vector.tensor_copy(
                            out_sb[:, k * 512 : (k + 1) * 512], ps_ps[:]
                        )
                for b in range(B):
                    nc.sync.dma_start(
                        out_d[b : b + 1, :],
                        out_sb[b : b + 1, b * N : (b + 1) * N],
                    )

            for rep in range(repeats + lag):
                if rep < repeats:
                    pre_stage(rep)
                if rep >= lag:
                    post_stage(rep - lag)

    nc.compile()
    return nc


def _get_nc():
    global _compiled_nc
    if _compiled_nc is None:
        _compiled_nc = _build_and_compile()
    return _compiled_nc


def make_in_maps(X, W_qkv):
    import ml_dtypes

    bf = ml_dtypes.bfloat16
    X = np.ascontiguousarray(X, dtype=np.float32)
    W = np.ascontiguousarray(W_qkv, dtype=np.float32)
    assert X.shape == (B, N, C) and W.shape == (2 * C, C)
    XT = np.ascontiguousarray(X.transpose(2, 0, 1).reshape(C, B * N)).astype(bf)
    maps = []
    for i in range(NCORES):
        ci = slice(i * JB, (i + 1) * JB)
        wkt = np.ascontiguousarray(W[C : 2 * C, ci].T).astype(bf)    # [96, 768]
        wq = W[0:C, ci]                                              # [768, 96]
        wq_sw = np.ascontiguousarray(
            wq.reshape(JT, 128, JB).transpose(1, 0, 2).reshape(128, JT * JB)
        ).astype(bf)
        maps.append(
            {
                "x_in": np.ascontiguousarray(XT[ci]),
                "wkt_in": wkt,
                "wq_in": wq_sw,
            }
        )
    return maps


def assemble_out(results):
    acc = results[0]["scores_out"].astype(np.float32).copy()
    for i in range(1, NCORES):
        acc += results[i]["scores_out"]
    return acc


def kernel(X, W_qkv):
    from concourse import bass_utils

    nc = _get_nc()
    res = bass_utils.run_bass_kernel_spmd(
        nc, make_in_maps(X, W_qkv), core_ids=list(range(NCORES))
    )
    return assemble_out(res.results)
